# revision 1
# baseline (speedup 1.0000x reference)
"""Trainium2 Bass kernel for nn_FeatureEmbedding (4-layer 3x3 conv CNN
with LeakyReLU + sinusoidal positional-encoding add).

Strategy
--------
Data-parallel over the batch dim: 32 batches x 12 frames = 384 images;
each of the 8 NeuronCores processes 48 images (4 batches). Per image
the whole layer chain runs out of SBUF; PSUM is managed as 4 rotating
double-bank [C, 1024] tiles; in the steady state the PE never idles
(cost-model occupancy ~95%, the rest is program startup/tail).

  - Input: host-staged im2col fold x9 [60, 4096] bf16 (tap row =
    kw*15 + kh*5 + cin for kw 0..3) -> ONE flat contiguous DMA per
    image, prefetched one image ahead on the SP queue. An on-device
    9-DMA gather cost ~5.6us of serialized HWDGE trigger time per
    image and stalled the PE at every image boundary; the host fold is
    a pure input-layout transform (like the weight marshaling below).
  - Layer 1 is 2-pixel-packed: stationary [60, 128] computes the even
    pixel's 64 channels in PE columns 0-63 and the odd pixel's in
    64-127, so 4 N=512 matmuls/image cover all pixels. Drains write
    interleaved even/odd columns; the odd-half drain reads PSUM
    partitions 64-127 into SBUF partitions 0-63 (validated exact on
    HW).
  - Layers 2-4 are shift-GEMM over zero-padded [C, 66*66] activation
    buffers: per 512-px column group the taps accumulate into one PSUM
    bank via strided moving windows. Layer 2 runs in 5 passes: 4
    paired K=128 taps against h1 = [A; A<<1] and h1x = [A<<64; A]
    pairing buffers (chunked SBUF-SBUF copies issued from the ACT
    queue right behind the L1 drains each chunk depends on), plus the
    (2,2) single K=64.
  - Layer 3 is tap-outer; layer 4 is tile-outer with the NEXT image's
    layer-1 tile emitted right behind each freed double-bank, so the
    cross-image pipeline never waits out a drain phase.
  - ScalarE drains every bank with fused Lrelu(psum + bias); layer 4
    drains to bf16 and the otherwise-idle DVE adds pe[:, t] per
    column group right behind each drain (the pe add must follow the
    nonlinearity, so it cannot fold into the bias). Output DMAs ride
    the SP queue (the ACT queue carries only the pairing copies, so
    neither engine head-of-line-blocks); the final image streams out
    per tile to shorten the program tail. The host casts bf16 -> f32.

All constants ship as 2 packed DMAs (one bf16 weight block, one f32
per-partition block) pre-marshaled on the host into the [K, M]
stationary layouts the PE wants. Two activation-buffer sets alternate
between images so DMA/PE/ACT pipeline across images.
"""

import numpy as np

import concourse.bass as bass
import concourse.bacc as bacc
import concourse.mybir as mybir
import concourse.tile as tile

F32 = mybir.dt.float32
BF16 = mybir.dt.bfloat16
AF = mybir.ActivationFunctionType

N_CORES = 8
B, T, CIN, H, W = 32, 12, 5, 64, 64
K1 = 60                # 12 tap rows x 5 cin: kw in 0..3 (2-px packing)
CH = [64, 128, 128, 128]
NPIX = H * W           # 4096
PITCH = W + 2          # 66 (padded row pitch for h buffers and xrow)
PAD = PITCH * PITCH    # 4356
NTILE = 8              # 512-pixel output tiles per image
RPT = H // NTILE       # 8 rows per tile
TILEPIX = RPT * W      # 512
ALPHA = 0.01           # LeakyReLU negative slope

TAPS = [(kh, kw) for kh in range(3) for kw in range(3)]

# packed-constant column offsets: w1 | w2n | w2r | w3 | w4
WOFF = [0, 2 * CH[0], 2 * CH[0] + 4 * CH[1],
        2 * CH[0] + 5 * CH[1], 2 * CH[0] + 5 * CH[1] + 9 * CH[2]]
WCOLS = WOFF[4] + 9 * CH[3]
FCOLS = 4 + T


def _build(nimg: int):
    """Build the per-core Bass program (SPMD: same program on all cores)."""
    nc = bacc.Bacc("TRN2", target_bir_lowering=False, debug=False)

    # x and weights are pre-cast to bf16 on the host, so all DMAs are
    # plain copies with no cast step.
    xin = nc.dram_tensor("xin", [nimg, K1, NPIX], BF16, kind="ExternalInput")
    # all bf16 weights packed into one [128, WCOLS] tensor, all f32
    # per-partition constants into one [128, FCOLS] tensor: 2 startup
    # DMAs instead of 10 serialized triggers
    wcd = nc.dram_tensor("wc", [2 * CH[0], WCOLS], BF16,
                         kind="ExternalInput")
    fcd = nc.dram_tensor("fc", [2 * CH[0], FCOLS], F32,
                         kind="ExternalInput")
    outd = nc.dram_tensor("out", [nimg, CH[3], NPIX], BF16,
                          kind="ExternalOutput")

    with tile.TileContext(nc) as tc:
        with (
            tc.tile_pool(name="wpool", bufs=1) as wp,
            tc.tile_pool(name="bpool", bufs=1) as bp,
            tc.tile_pool(name="psum", bufs=4, space="PSUM") as pp,
        ):
            # --- constants ---
            wcs = wp.tile([2 * CH[0], WCOLS], BF16)
            nc.sync.dma_start(out=wcs, in_=wcd[:, :])
            fcs = wp.tile([2 * CH[0], FCOLS], F32)
            nc.sync.dma_start(out=fcs, in_=fcd[:, :])
            w1s = wcs[0:K1, 0:2 * CH[0]]
            w2ns = wcs[:, WOFF[1]:WOFF[1] + 4 * CH[1]]
            w2rs = wcs[0:CH[0], WOFF[2]:WOFF[2] + CH[1]]
            w3s = wcs[:, WOFF[3]:WOFF[3] + 9 * CH[2]]
            w4s = wcs[:, WOFF[4]:WOFF[4] + 9 * CH[3]]
            b1s = fcs[:, 0:1]
            b2s = fcs[:, 1:2]
            b3s = fcs[:, 2:3]
            b4s = fcs[:, 3:4]
            pes = fcs[:, 4:4 + T]

            # --- persistent activation buffers, double-buffered ---
            sets = []
            for s in range(2):
                x9 = bp.tile([K1, NPIX], BF16, name=f"x9_{s}")
                # h1 holds copy A (parts 0-63) and copy B (parts 64-127,
                # shifted one element left in flat padded space; the wrap
                # lands only on halo zeros, so one contiguous SBUF->SBUF
                # DMA produces an exact shifted-padded copy for pairing
                # layer 2's kw in {0,1} taps into K=128 matmuls)
                h1 = bp.tile([2 * CH[0], PAD], BF16, name=f"h1_{s}")
                h1x = bp.tile([2 * CH[0], PAD], BF16, name=f"h1x_{s}")
                h2 = bp.tile([CH[1], PAD], BF16, name=f"h2_{s}")
                h3 = bp.tile([CH[2], PAD], BF16, name=f"h3_{s}")
                h4 = bp.tile([CH[3], NPIX], BF16, name=f"h4_{s}")
                # one-time zeroing of the halo ring (the only padded-buffer
                # cells the drains never rewrite): top/bottom rows and
                # left/right columns
                for buf, np_ in ((h1, CH[0]), (h2, CH[1]), (h3, CH[2])):
                    v = buf.rearrange("p (r c) -> p r c", c=PITCH)
                    u = mybir.dt.uint16
                    nc.vector.memset(v[0:np_, 0:1, :].bitcast(u), 0.0)
                    nc.vector.memset(
                        v[0:np_, PITCH - 1:PITCH, :].bitcast(u), 0.0)
                    nc.vector.memset(
                        v[0:np_, 1:PITCH - 1, 0:1].bitcast(u), 0.0)
                    nc.vector.memset(
                        v[0:np_, 1:PITCH - 1, PITCH - 1:PITCH].bitcast(u),
                        0.0)
                sets.append((x9, h1, h1x, h2, h3, h4))

            # preload x9 for image 0 and run its layer 1 standalone
            # (for img >= 1, layer-1 matmuls are interleaved into the
            # previous image's layer-4 tile loop so they reuse each
            # freed PSUM double-bank immediately)
            nc.sync.dma_start(out=sets[0][0], in_=xin[0])

            def l1_tile(img, j):
                """2-px-packed layer-1 tile j (of 2): one [128, 1024]
                PSUM double-bank covers 2048 pixels -- stationary cols
                0-63 are the even pixel's channels, 64-127 the odd
                pixel's. Drains write interleaved columns; the odd-half
                drain reads PSUM partitions 64-127 into SBUF partitions
                0-63 (validated on HW)."""
                x9 = sets[img % 2][0]
                xv = x9.rearrange("p (n two) -> p n two", two=2)
                ps = pp.tile([2 * CH[0], 2 * TILEPIX], F32,
                             name=f"ps1_{img}_{j}", tag="ps")
                for hh in range(2):
                    c0 = j * 2 * TILEPIX + hh * TILEPIX
                    nc.tensor.matmul(
                        ps[:, hh * TILEPIX:(hh + 1) * TILEPIX],
                        w1s, xv[:, c0:c0 + TILEPIX, 0:1],
                        start=True, stop=True)
                h1 = sets[img % 2][1]
                h1q = h1.rearrange("p (r ct two) -> p r ct two", ct=33,
                                   two=2)
                r0 = 1 + 32 * j
                nc.scalar.activation(
                    h1q[0:CH[0], r0:r0 + 32, 0:32, 1:2], ps[0:CH[0], :],
                    AF.Lrelu, bias=b1s[0:CH[0], 0:1], scale=1.0,
                    alpha=ALPHA)
                nc.scalar.activation(
                    h1q[0:CH[0], r0:r0 + 32, 1:33, 0:1],
                    ps[CH[0]:2 * CH[0], :], AF.Lrelu,
                    bias=b1s[CH[0]:2 * CH[0], 0:1], scale=1.0, alpha=ALPHA)
                # copy-B chunk + pairing-buffer h1x chunks (ACT queue;
                # depend only on the two drains just above)
                h1x = sets[img % 2][2]
                mid = PITCH * 33 - 1
                cb = (0, mid, PAD - 1)
                nc.scalar.dma_start(
                    out=h1[CH[0]:2 * CH[0], cb[j]:cb[j + 1]],
                    in_=h1[0:CH[0], cb[j] + 1:cb[j + 1] + 1])
                if j == 0:
                    nc.scalar.dma_start(out=h1x[0:CH[0], 0:mid - W],
                                        in_=h1[0:CH[0], W:mid])
                    nc.scalar.dma_start(out=h1x[CH[0]:2 * CH[0], 0:mid],
                                        in_=h1[0:CH[0], 0:mid])
                else:
                    nc.scalar.dma_start(out=h1x[0:CH[0], mid - W:PAD - W],
                                        in_=h1[0:CH[0], mid:PAD])
                    nc.scalar.dma_start(out=h1x[CH[0]:2 * CH[0], mid:PAD],
                                        in_=h1[0:CH[0], mid:PAD])

            for j in range(2):
                l1_tile(0, j)

            for img in range(nimg):
                x9, h1, h1x, h2, h3, h4 = sets[img % 2]
                t = img % T
                h1v = h1.rearrange("p (r c) -> p r c", c=PITCH)
                h1xv = h1x.rearrange("p (r c) -> p r c", c=PITCH)
                h2v = h2.rearrange("p (r c) -> p r c", c=PITCH)
                h3v = h3.rearrange("p (r c) -> p r c", c=PITCH)

                # prefetch next image's x9 (the SP queue carries only
                # these and the output stores)
                if img + 1 < nimg:
                    nc.sync.dma_start(out=sets[(img + 1) % 2][0],
                                      in_=xin[img + 1])

                # layer 2 in 5 passes: per column group, the single tap
                # (2,2) K=64 then paired K=128 passes (0,0)+(0,1),
                # (1,1)+(1,2), (2,0)+(2,1) -- grouped so each group's 4
                # matmuls fire together as soon as its L1 drains exist.
                # The (1,0)+(0,2) pass against h1x runs last (it waits
                # on the h1x copies).
                ps2 = [pp.tile([CH[1], 2 * TILEPIX], F32,
                               name=f"ps2_{img}_{j}", tag="ps")
                       for j in range(NTILE // 2)]
                a128 = slice(0, 2 * CH[0])
                a64 = slice(0, CH[0])
                for j in range(NTILE // 2):
                    for hh in range(2):
                        r0 = (2 * j + hh) * RPT
                        oslc = slice(hh * TILEPIX, (hh + 1) * TILEPIX)
                        for wap, view, p0, kh0, c0, start in (
                            (w2rs, h1v, a64, 2, 2, True),
                            (w2ns[:, 0:CH[1]], h1v, a128, 0, 0, False),
                            (w2ns[:, 2 * CH[1]:3 * CH[1]], h1v, a128,
                             1, 1, False),
                        ):
                            nc.tensor.matmul(
                                ps2[j][:, oslc], wap,
                                view[p0, r0 + kh0:r0 + kh0 + RPT,
                                     c0:c0 + W],
                                start=start, stop=False)
                for j in range(NTILE // 2):
                    for hh in range(2):
                        r0 = (2 * j + hh) * RPT
                        nc.tensor.matmul(
                            ps2[j][:, hh * TILEPIX:(hh + 1) * TILEPIX],
                            w2ns[:, 3 * CH[1]:4 * CH[1]],
                            h1v[:, r0 + 2:r0 + 2 + RPT, 0:W],
                            start=False, stop=False)
                for j in range(NTILE // 2):
                    for hh in range(2):
                        r0 = (2 * j + hh) * RPT
                        nc.tensor.matmul(
                            ps2[j][:, hh * TILEPIX:(hh + 1) * TILEPIX],
                            w2ns[:, CH[1]:2 * CH[1]],
                            h1xv[:, r0:r0 + RPT, 2:2 + W],
                            start=False, stop=True)
                    r0 = j * 2 * RPT
                    nc.scalar.activation(
                        h2v[:, 1 + r0:1 + r0 + 2 * RPT, 1:1 + W], ps2[j],
                        AF.Lrelu, bias=b2s[:, 0:1], scale=1.0, alpha=ALPHA)

                # layer 3: tap-outer (weight loads amortize over all
                # 8 column groups)
                ps3 = [pp.tile([CH[2], 2 * TILEPIX], F32,
                               name=f"ps3_{img}_{j}", tag="ps")
                       for j in range(NTILE // 2)]
                for ti, (kh, kw) in enumerate(TAPS):
                    tap = kh * 3 + kw
                    for j in range(NTILE // 2):
                        for hh in range(2):
                            r0 = (2 * j + hh) * RPT
                            nc.tensor.matmul(
                                ps3[j][:, hh * TILEPIX:(hh + 1) * TILEPIX],
                                w3s[:, tap * CH[2]:(tap + 1) * CH[2]],
                                h2v[:, r0 + kh:r0 + kh + RPT, kw:kw + W],
                                start=(ti == 0), stop=(ti == 8))
                        if ti == 8:
                            r0 = j * 2 * RPT
                            nc.scalar.activation(
                                h3v[:, 1 + r0:1 + r0 + 2 * RPT, 1:1 + W],
                                ps3[j], AF.Lrelu, bias=b3s[:, 0:1],
                                scale=1.0, alpha=ALPHA)

                # layer 4, tile-outer, with the NEXT image's layer-1
                # tile pipelined right behind each freed double-bank
                for j in range(NTILE // 2):
                    ps4 = pp.tile([CH[3], 2 * TILEPIX], F32,
                                  name=f"ps4_{img}_{j}", tag="ps")
                    for ti, (kh, kw) in enumerate(TAPS):
                        tap = kh * 3 + kw
                        for hh in range(2):
                            r0 = (2 * j + hh) * RPT
                            nc.tensor.matmul(
                                ps4[:, hh * TILEPIX:(hh + 1) * TILEPIX],
                                w4s[:, tap * CH[3]:(tap + 1) * CH[3]],
                                h3v[:, r0 + kh:r0 + kh + RPT, kw:kw + W],
                                start=(ti == 0), stop=(ti == 8))
                    seg = h4[:, j * 2 * TILEPIX:(j + 1) * 2 * TILEPIX]
                    nc.scalar.activation(
                        seg, ps4, AF.Lrelu, bias=b4s[:, 0:1], scale=1.0,
                        alpha=ALPHA)
                    nc.vector.tensor_scalar_add(seg, seg, pes[:, t:t + 1])
                    if img + 1 < nimg and j in (0, 2):
                        l1_tile(img + 1, j // 2)

                # store (SP queue: waits here without blocking ACT).
                # The final image streams out per L4 tile so the
                # program tail is one chunk, not a whole-image DMA.
                if img + 1 < nimg:
                    nc.sync.dma_start(out=outd[img], in_=h4)
                else:
                    for j in range(NTILE // 2):
                        s0 = j * 2 * TILEPIX
                        nc.sync.dma_start(
                            out=outd[img, :, s0:s0 + 2 * TILEPIX],
                            in_=h4[:, s0:s0 + 2 * TILEPIX])

    nc.compile()
    return nc


def _pe_table():
    d = np.arange(CH[3])
    d_even = (d // 2) * 2
    tt = np.arange(T, dtype=np.float64)
    arg = tt[:, None] / np.power(10000.0, d_even / CH[3])
    pe = np.where(d % 2 == 0, np.sin(arg), np.cos(arg))  # [T, D]
    return np.ascontiguousarray(pe.T.astype(np.float32))  # [D, T]


def _cast_bf16(a):
    """fp32 -> bf16 (round-to-nearest-even) on the host so on-chip DMAs
    are plain copies."""
    import ml_dtypes
    return np.ascontiguousarray(np.asarray(a, dtype=np.float32)).astype(
        ml_dtypes.bfloat16)


def _w1_layout(w0):
    """[K1, 128] 2-px stationary: cols 0-63 = even pixel's channels
    (tap rows kw 0-2), cols 64-127 = odd pixel's (tap rows kw 1-3 hold
    its kw-1 weights)."""
    out = np.zeros((K1, 2 * CH[0]), dtype=np.float32)
    for kw in range(3):
        for kh in range(3):
            base = kw * 15 + kh * CIN
            out[base:base + CIN, 0:CH[0]] = w0[:, :, kh, kw].T
            out[base + 15:base + 15 + CIN, CH[0]:2 * CH[0]] = \
                w0[:, :, kh, kw].T
    return out


def _w2_passes(w1):
    """[128, 4*128]: per pass block, rows 0-63 = the tap read from the
    pairing buffer's low half, rows 64-127 = its high half. Blocks:
    (0,0)+(0,1), (1,0)+(0,2), (1,1)+(1,2), (2,0)+(2,1)."""
    a = w1.transpose(1, 2, 3, 0)  # [cin, kh, kw, cout]
    blocks = [((0, 0), (0, 1)), ((1, 0), (0, 2)),
              ((1, 1), (1, 2)), ((2, 0), (2, 1))]
    cols = [np.concatenate([a[:, lo[0], lo[1], :], a[:, hi[0], hi[1], :]],
                           axis=0) for lo, hi in blocks]
    return np.ascontiguousarray(np.concatenate(cols, axis=1))


def _prep_consts(w0, b0, w1, b1, w2, b2, w3, b3):
    wc = np.zeros((2 * CH[0], WCOLS), dtype=np.float32)
    wc[0:K1, 0:2 * CH[0]] = _w1_layout(np.asarray(w0))
    wc[:, WOFF[1]:WOFF[1] + 4 * CH[1]] = _w2_passes(np.asarray(w1))
    wc[0:CH[0], WOFF[2]:WOFF[2] + CH[1]] = np.ascontiguousarray(
        np.asarray(w1).transpose(1, 2, 3, 0)[:, 2, 2, :])
    wc[:, WOFF[3]:WOFF[3] + 9 * CH[2]] = np.asarray(w2).transpose(
        1, 2, 3, 0).reshape(CH[1], 9 * CH[2])
    wc[:, WOFF[4]:WOFF[4] + 9 * CH[3]] = np.asarray(w3).transpose(
        1, 2, 3, 0).reshape(CH[2], 9 * CH[3])
    fc = np.zeros((2 * CH[0], FCOLS), dtype=np.float32)
    fc[:, 0] = np.tile(np.asarray(b0, dtype=np.float32), 2)
    fc[:, 1] = np.asarray(b1, dtype=np.float32)
    fc[:, 2] = np.asarray(b2, dtype=np.float32)
    fc[:, 3] = np.asarray(b3, dtype=np.float32)
    fc[:, 4:4 + T] = _pe_table()
    return {"wc": _cast_bf16(wc), "fc": np.ascontiguousarray(fc)}


_prog_cache: dict[int, object] = {}


def _get_program(nimg: int):
    if nimg not in _prog_cache:
        _prog_cache[nimg] = _build(nimg)
    return _prog_cache[nimg]


_runner_cache: dict[int, object] = {}


def _get_runner(nimg: int):
    """A reusable jitted SPMD executor for the per-core program (avoids
    re-tracing/re-lowering on every kernel() call)."""
    if nimg in _runner_cache:
        return _runner_cache[nimg]

    import jax
    import jax.numpy as jnp
    from concourse.bass2jax import (
        install_neuronx_cc_hook, partition_id_tensor, _bass_exec_p)
    from jax.sharding import Mesh, PartitionSpec, NamedSharding
    from jax.experimental.shard_map import shard_map

    nc = _get_program(nimg)
    install_neuronx_cc_hook()

    partition_name = (nc.partition_id_tensor.name
                      if nc.partition_id_tensor else None)
    in_names, out_names, out_avals, zero_shapes = [], [], [], []
    for alloc in nc.m.functions[0].allocations:
        if not isinstance(alloc, mybir.MemoryLocationSet):
            continue
        name = alloc.memorylocations[0].name
        if alloc.kind == "ExternalInput":
            if name != partition_name:
                in_names.append(name)
        elif alloc.kind == "ExternalOutput":
            shape = tuple(alloc.tensor_shape)
            dtype = mybir.dt.np(alloc.dtype)
            out_names.append(name)
            out_avals.append(jax.core.ShapedArray(shape, dtype))
            zero_shapes.append((shape, dtype))
    n_params = len(in_names)
    n_outs = len(out_names)
    all_in_names = list(in_names) + list(out_names)
    if partition_name is not None:
        all_in_names.append(partition_name)

    def _body(*args):
        operands = list(args)
        if partition_name is not None:
            operands.append(partition_id_tensor())
        outs = _bass_exec_p.bind(
            *operands,
            out_avals=tuple(out_avals),
            in_names=tuple(all_in_names),
            out_names=tuple(out_names),
            lowering_input_output_aliases=(),
            sim_require_finite=True,
            sim_require_nnan=True,
            nc=nc,
        )
        return tuple(outs)

    devices = jax.devices()[:N_CORES]
    mesh = Mesh(np.asarray(devices), ("core",))
    sh = NamedSharding(mesh, PartitionSpec("core"))
    donate = tuple(range(n_params, n_params + n_outs))
    sharded = jax.jit(
        shard_map(_body, mesh=mesh,
                  in_specs=(PartitionSpec("core"),) * (n_params + n_outs),
                  out_specs=(PartitionSpec("core"),) * n_outs,
                  check_rep=False),
        donate_argnums=donate, keep_unused=True)
    zeros_fn = jax.jit(
        lambda: tuple(
            jnp.zeros((N_CORES * s[0], *s[1:]), d) for s, d in zero_shapes),
        out_shardings=(sh,) * n_outs)

    def run(in_maps):
        concat_in = [
            np.concatenate([np.asarray(in_maps[c][nm])
                            for c in range(N_CORES)], axis=0)
            for nm in in_names
        ]
        dev_in = [jax.device_put(a, sh) for a in concat_in]
        outs = sharded(*dev_in, *zeros_fn())
        oi = out_names.index("out")
        return np.asarray(outs[oi])

    _runner_cache[nimg] = run
    return run


def _stage_x(x_core):
    """[n, CIN, H, W] f32 -> host im2col fold: [n, K1, NPIX] bf16 where
    row kw*15 + kh*5 + c at column (r, col) holds the zero-padded input
    value x[c, r + kh - 1, col + kw - 1]; kw=3 is the extra tap column
    the odd pixel of each packed pair needs."""
    n = x_core.shape[0]
    xp = np.zeros((n, CIN, H + 2, W + 3), dtype=np.float32)
    xp[:, :, 1:1 + H, 1:1 + W] = x_core
    x9 = np.empty((n, K1, NPIX), dtype=np.float32)
    for kw in range(4):
        for kh in range(3):
            base = kw * 15 + kh * CIN
            x9[:, base:base + CIN] = xp[:, :, kh:kh + H, kw:kw + W].reshape(
                n, CIN, NPIX)
    return _cast_bf16(x9)


def make_in_maps(x, w0, b0, w1, b1, w2, b2, w3, b3):
    """Shard the full inputs into the 8 per-core input maps."""
    consts = _prep_consts(w0, b0, w1, b1, w2, b2, w3, b3)
    bpc = B // N_CORES  # batches per core
    in_maps = []
    for c in range(N_CORES):
        xs = _stage_x(
            np.asarray(x)[c * bpc:(c + 1) * bpc].reshape(
                bpc * T, CIN, H, W))
        in_maps.append({"xin": xs, **consts})
    return in_maps


def kernel(x, w0, b0, w1, b1, w2, b2, w3, b3):
    nimg = (B // N_CORES) * T
    run = _get_runner(nimg)
    in_maps = make_in_maps(x, w0, b0, w1, b1, w2, b2, w3, b3)
    glob = run(in_maps)  # [8*nimg, 128, 4096] bf16
    bpc = B // N_CORES
    out = glob.reshape(N_CORES * bpc, T, CH[3], H, W).reshape(
        B, T, CH[3], H, W)
    return np.ascontiguousarray(out.astype(np.float32))



# revision 20
# speedup vs baseline: 4.8594x; 4.8594x over previous
"""Trainium2 Bass kernel for nn_FeatureEmbedding (4-layer 3x3 conv CNN
with LeakyReLU + sinusoidal positional-encoding add).

Strategy
--------
Data-parallel over the batch dim: 32 batches x 12 frames = 384 images;
each of the 8 NeuronCores processes 48 images (4 batches). Per image
the whole layer chain runs out of SBUF.

PSUM is managed as 8 rotating SINGLE-bank [C, 512] tiles (one bank
each) with per-bank ScalarE drains. Profiling the double-bank variant
showed ~500 matmuls/run stalling ~0.4-1.1us each on bank-recycle
semaphores (drain latency of [128,1024] ACT reads); single-bank
granularity halves the drain latency and doubles the rotation slack,
so the PE stream stays within ~2% of the pure-matmul floor.

  - Input: host-staged im2col fold x9 [60, 4096] bf16 -> ONE flat DMA
    per image, prefetched one image ahead on the SP queue. Startup
    order: w1 -> biases/pe -> x9[0] -> bulk weights, so layer 1
    starts ~3us earlier than with one packed constant DMA.
  - Layer 1 is 2-pixel-packed: stationary [60, 128] computes the even
    pixel's 64 channels in PE columns 0-63 and the odd pixel's in
    64-127; 4 N=512 matmuls/image (one per bank, 16 image rows each).
    Per-bank drains write interleaved even/odd columns; the odd-half
    drain reads PSUM partitions 64-127 into SBUF partitions 0-63.
  - Layers 2-4 are shift-GEMM over zero-padded [C, 66*66] activation
    buffers, one 512-pixel (8-row) column group per PSUM bank.
  - Layer 2 runs per bank-pair: the (2,2) K=64 tap for the even bank
    (weights on PE rows 0-63 vs h1 copy A) and the odd bank (weights
    on rows 64-127 vs the shifted copy B) are issued back-to-back as
    row-tiled matmuls that execute CONCURRENTLY in the PE array
    (tile_position row groups), halving the K=64 tap's cost. The 4
    paired K=128 taps then run against h1 = [A; A<<1] and h1x =
    [A<<64; A] pairing buffers (chunked SBUF-SBUF copies on the ACT
    queue, issued right behind the L1 drains each chunk depends on).
  - Layers 3-4 are bank-outer (9 taps accumulate back-to-back, then
    the bank drains while the next bank computes), which staggers the
    drains evenly instead of queueing 4 of them at each layer edge.
  - ScalarE drains every bank with fused Lrelu(psum + bias); layer 4
    drains to bf16 and the otherwise-idle DVE adds pe[:, t] per bank
    right behind each drain. The NEXT image's layer-1 banks are
    emitted behind every other freed L4 bank so the cross-image
    pipeline never waits out a drain phase. Output DMAs ride the SP
    queue; the final image streams out per bank to shorten the tail.
    The host casts bf16 -> f32.

Constants ship as 4 DMAs (w1, biases+pe, then the bulk bf16 weight
block) pre-marshaled on the host into the [K, M] stationary layouts
the PE wants. Two activation-buffer sets alternate between images so
DMA/PE/ACT pipeline across images.
"""

import numpy as np

import concourse.bass as bass
import concourse.bacc as bacc
import concourse.mybir as mybir
import concourse.tile as tile

F32 = mybir.dt.float32
BF16 = mybir.dt.bfloat16
AF = mybir.ActivationFunctionType

N_CORES = 8
B, T, CIN, H, W = 32, 12, 5, 64, 64
K1 = 60                # 12 tap rows x 5 cin: kw in 0..3 (2-px packing)
CH = [64, 128, 128, 128]
NPIX = H * W           # 4096
PITCH = W + 2          # 66 (padded row pitch for h buffers)
PAD = PITCH * PITCH    # 4356
NBANK = 8              # 512-pixel column groups (one PSUM bank each)
RPT = H // NBANK       # 8 rows per bank group
TILEPIX = RPT * W      # 512
ALPHA = 0.01           # LeakyReLU negative slope

TAPS = [(kh, kw) for kh in range(3) for kw in range(3)]

# packed-constant column offsets in wc: w2n | w22 | w3 | w4
WOFF = [0, 4 * CH[1], 5 * CH[1], 5 * CH[1] + 9 * CH[2]]
WCOLS = WOFF[3] + 9 * CH[3]
FCOLS = 4 + T

# L1 bank boundaries in padded flat space (4 banks x 16 rows)
CB = (0, PITCH * 17 - 1, PITCH * 33 - 1, PITCH * 49 - 1, PAD - 1)
SBL = (0, CB[1] - W, CB[2] - W, CB[3] - W, PAD - W)


def _build(nimg: int):
    """Build the per-core Bass program (SPMD: same program on all cores)."""
    nc = bacc.Bacc("TRN2", target_bir_lowering=False, debug=False)

    xin = nc.dram_tensor("xin", [nimg, K1, NPIX], BF16, kind="ExternalInput")
    w1d = nc.dram_tensor("w1c", [K1, 2 * CH[0]], BF16, kind="ExternalInput")
    wcd = nc.dram_tensor("wc", [2 * CH[0], WCOLS], BF16,
                         kind="ExternalInput")
    fcd = nc.dram_tensor("fc", [2 * CH[0], FCOLS], F32,
                         kind="ExternalInput")
    outd = nc.dram_tensor("out", [nimg, CH[3], NPIX], BF16,
                          kind="ExternalOutput")

    with tile.TileContext(nc) as tc:
        with (
            tc.tile_pool(name="wpool", bufs=1) as wp,
            tc.tile_pool(name="bpool", bufs=1) as bp,
            tc.tile_pool(name="psum", bufs=8, space="PSUM") as pp,
        ):
            # --- constants (SP queue order = need order) ---
            w1s = wp.tile([K1, 2 * CH[0]], BF16)
            nc.sync.dma_start(out=w1s, in_=w1d[:, :])
            fcs = wp.tile([2 * CH[0], FCOLS], F32)
            nc.sync.dma_start(out=fcs, in_=fcd[:, :])
            # x9 for image 0 goes next (sets built below), then bulk wc
            wcs = wp.tile([2 * CH[0], WCOLS], BF16)

            w2ns = wcs[:, WOFF[0]:WOFF[0] + 4 * CH[1]]
            w22s = wcs[:, WOFF[1]:WOFF[1] + CH[1]]
            w3s = wcs[:, WOFF[2]:WOFF[2] + 9 * CH[2]]
            w4s = wcs[:, WOFF[3]:WOFF[3] + 9 * CH[3]]
            b1s = fcs[:, 0:1]
            b2s = fcs[:, 1:2]
            b3s = fcs[:, 2:3]
            b4s = fcs[:, 3:4]
            pes = fcs[:, 4:4 + T]

            # --- persistent activation buffers, double-buffered ---
            sets = []
            for s in range(2):
                x9 = bp.tile([K1, NPIX], BF16, name=f"x9_{s}")
                h1 = bp.tile([2 * CH[0], PAD], BF16, name=f"h1_{s}")
                h1x = bp.tile([2 * CH[0], PAD], BF16, name=f"h1x_{s}")
                h2 = bp.tile([CH[1], PAD], BF16, name=f"h2_{s}")
                h3 = bp.tile([CH[2], PAD], BF16, name=f"h3_{s}")
                h4 = bp.tile([CH[3], NPIX], BF16, name=f"h4_{s}")
                # one-time zeroing of the halo ring
                for buf, np_ in ((h1, CH[0]), (h2, CH[1]), (h3, CH[2])):
                    v = buf.rearrange("p (r c) -> p r c", c=PITCH)
                    u = mybir.dt.uint16
                    nc.vector.memset(v[0:np_, 0:1, :].bitcast(u), 0.0)
                    nc.vector.memset(
                        v[0:np_, PITCH - 1:PITCH, :].bitcast(u), 0.0)
                    nc.vector.memset(
                        v[0:np_, 1:PITCH - 1, 0:1].bitcast(u), 0.0)
                    nc.vector.memset(
                        v[0:np_, 1:PITCH - 1, PITCH - 1:PITCH].bitcast(u),
                        0.0)
                sets.append((x9, h1, h1x, h2, h3, h4))

            # preload x9 for image 0, then the bulk weights
            nc.sync.dma_start(out=sets[0][0], in_=xin[0])
            nc.sync.dma_start(out=wcs, in_=wcd[:, :])

            def l1_bank(img, j):
                """2-px-packed layer-1 bank j (of 4): one [128, 512] PSUM
                bank covers 1024 pixels (16 image rows) -- stationary
                cols 0-63 are the even pixel's channels, 64-127 the odd
                pixel's. Copy-B / h1x pairing chunks (one double-size set
                per bank pair, 6 copies/image) are issued on the ACT
                queue right behind the drains they depend on."""
                x9 = sets[img % 2][0]
                xv = x9.rearrange("p (n two) -> p n two", two=2)
                ps = pp.tile([2 * CH[0], TILEPIX], F32,
                             name=f"ps1_{img}_{j}", tag="ps")
                c0 = j * TILEPIX
                nc.tensor.matmul(ps, w1s, xv[:, c0:c0 + TILEPIX, 0:1],
                                 start=True, stop=True)
                h1 = sets[img % 2][1]
                h1q = h1.rearrange("p (r ct two) -> p r ct two", ct=33,
                                   two=2)
                r0 = 1 + 16 * j
                nc.scalar.activation(
                    h1q[0:CH[0], r0:r0 + 16, 0:32, 1:2], ps[0:CH[0], :],
                    AF.Lrelu, bias=b1s[0:CH[0], 0:1], scale=1.0,
                    alpha=ALPHA)
                nc.scalar.activation(
                    h1q[0:CH[0], r0:r0 + 16, 1:33, 0:1],
                    ps[CH[0]:2 * CH[0], :], AF.Lrelu,
                    bias=b1s[CH[0]:2 * CH[0], 0:1], scale=1.0, alpha=ALPHA)
                if j in (1, 3):
                    h1x = sets[img % 2][2]
                    c0_, c1_ = CB[j - 1], CB[j + 1]
                    s0_, s1_ = SBL[j - 1], SBL[j + 1]
                    nc.scalar.dma_start(
                        out=h1[CH[0]:2 * CH[0], c0_:c1_],
                        in_=h1[0:CH[0], c0_ + 1:c1_ + 1])
                    nc.scalar.dma_start(
                        out=h1x[0:CH[0], s0_:s1_],
                        in_=h1[0:CH[0], s0_ + W:s1_ + W])
                    nc.scalar.dma_start(
                        out=h1x[CH[0]:2 * CH[0], c0_:c1_],
                        in_=h1[0:CH[0], c0_:c1_])

            for j in range(4):
                l1_bank(0, j)

            for img in range(nimg):
                x9, h1, h1x, h2, h3, h4 = sets[img % 2]
                t = img % T
                h1v = h1.rearrange("p (r c) -> p r c", c=PITCH)
                h1xv = h1x.rearrange("p (r c) -> p r c", c=PITCH)
                h2v = h2.rearrange("p (r c) -> p r c", c=PITCH)
                h3v = h3.rearrange("p (r c) -> p r c", c=PITCH)

                # prefetch next image's x9
                if img + 1 < nimg:
                    nc.sync.dma_start(out=sets[(img + 1) % 2][0],
                                      in_=xin[img + 1])

                # layer 2 per bank-pair: 3 paired K=128 passes, then the
                # (2,2) K=64 taps of both banks back-to-back as row-tiled
                # matmuls that run CONCURRENTLY in the PE (copy A on PE
                # rows 0-63, shifted copy B on rows 64-127), then the h1x
                # pass last (it waits on the latest pairing copies).
                a128 = slice(0, 2 * CH[0])
                for jp in range(4):
                    pa = pp.tile([CH[1], TILEPIX], F32,
                                 name=f"ps2_{img}_{2 * jp}", tag="ps")
                    pb = pp.tile([CH[1], TILEPIX], F32,
                                 name=f"ps2_{img}_{2 * jp + 1}", tag="ps")
                    ra = 2 * jp * RPT
                    rb = ra + RPT
                    for wap, view, kh0, c0, first in (
                        (w2ns[:, 0:CH[1]], h1v, 0, 0, True),
                        (w2ns[:, 2 * CH[1]:3 * CH[1]], h1v, 1, 1, False),
                        (w2ns[:, 3 * CH[1]:4 * CH[1]], h1v, 2, 0, False),
                    ):
                        for ps_, r0 in ((pa, ra), (pb, rb)):
                            nc.tensor.matmul(
                                ps_, wap,
                                view[a128, r0 + kh0:r0 + kh0 + RPT,
                                     c0:c0 + W],
                                start=first, stop=False)
                    # (2,2) K=64 concurrent row-tiled pair (copy-B dep)
                    nc.tensor.matmul(
                        pa, w22s[0:CH[0], :],
                        h1v[0:CH[0], ra + 2:ra + 2 + RPT, 2:2 + W],
                        start=False, stop=False)
                    nc.tensor.matmul(
                        pb, w22s[CH[0]:2 * CH[0], :],
                        h1v[CH[0]:2 * CH[0], rb + 2:rb + 2 + RPT, 1:1 + W],
                        start=False, stop=False)
                    for ps_, r0 in ((pa, ra), (pb, rb)):
                        nc.tensor.matmul(
                            ps_, w2ns[:, CH[1]:2 * CH[1]],
                            h1xv[a128, r0:r0 + RPT, 2:2 + W],
                            start=False, stop=True)
                    for ps_, r0 in ((pa, ra), (pb, rb)):
                        nc.scalar.activation(
                            h2v[:, 1 + r0:1 + r0 + RPT, 1:1 + W], ps_,
                            AF.Lrelu, bias=b2s[:, 0:1], scale=1.0,
                            alpha=ALPHA)

                # layers 3 and 4: bank-outer, drains staggered behind
                # each bank's 9-tap accumulation
                for g in range(NBANK):
                    ps3 = pp.tile([CH[2], TILEPIX], F32,
                                  name=f"ps3_{img}_{g}", tag="ps")
                    r0 = g * RPT
                    for ti, (kh, kw) in enumerate(TAPS):
                        tap = kh * 3 + kw
                        nc.tensor.matmul(
                            ps3, w3s[:, tap * CH[2]:(tap + 1) * CH[2]],
                            h2v[:, r0 + kh:r0 + kh + RPT, kw:kw + W],
                            start=(ti == 0), stop=(ti == 8))
                    nc.scalar.activation(
                        h3v[:, 1 + r0:1 + r0 + RPT, 1:1 + W], ps3,
                        AF.Lrelu, bias=b3s[:, 0:1], scale=1.0, alpha=ALPHA)

                for g in range(NBANK):
                    ps4 = pp.tile([CH[3], TILEPIX], F32,
                                  name=f"ps4_{img}_{g}", tag="ps")
                    r0 = g * RPT
                    for ti, (kh, kw) in enumerate(TAPS):
                        tap = kh * 3 + kw
                        nc.tensor.matmul(
                            ps4, w4s[:, tap * CH[3]:(tap + 1) * CH[3]],
                            h3v[:, r0 + kh:r0 + kh + RPT, kw:kw + W],
                            start=(ti == 0), stop=(ti == 8))
                    seg = h4[:, g * TILEPIX:(g + 1) * TILEPIX]
                    nc.scalar.activation(
                        seg, ps4, AF.Lrelu, bias=b4s[:, 0:1], scale=1.0,
                        alpha=ALPHA)
                    nc.vector.tensor_scalar_add(seg, seg, pes[:, t:t + 1])
                    if img + 1 < nimg and g < 4:
                        l1_bank(img + 1, g)
                    if img + 1 >= nimg:
                        nc.sync.dma_start(
                            out=outd[img, :, g * TILEPIX:(g + 1) * TILEPIX],
                            in_=seg)

                # store (SP queue; the final image streamed out per bank
                # above so the program tail is one chunk)
                if img + 1 < nimg:
                    nc.sync.dma_start(out=outd[img], in_=h4)

    nc.compile()
    return nc


def _pe_table():
    d = np.arange(CH[3])
    d_even = (d // 2) * 2
    tt = np.arange(T, dtype=np.float64)
    arg = tt[:, None] / np.power(10000.0, d_even / CH[3])
    pe = np.where(d % 2 == 0, np.sin(arg), np.cos(arg))  # [T, D]
    return np.ascontiguousarray(pe.T.astype(np.float32))  # [D, T]


def _cast_bf16(a):
    """fp32 -> bf16 (round-to-nearest-even) on the host so on-chip DMAs
    are plain copies."""
    import ml_dtypes
    return np.ascontiguousarray(np.asarray(a, dtype=np.float32)).astype(
        ml_dtypes.bfloat16)


def _w1_layout(w0):
    """[K1, 128] 2-px stationary: cols 0-63 = even pixel's channels
    (tap rows kw 0-2), cols 64-127 = odd pixel's (tap rows kw 1-3 hold
    its kw-1 weights)."""
    out = np.zeros((K1, 2 * CH[0]), dtype=np.float32)
    for kw in range(3):
        for kh in range(3):
            base = kw * 15 + kh * CIN
            out[base:base + CIN, 0:CH[0]] = w0[:, :, kh, kw].T
            out[base + 15:base + 15 + CIN, CH[0]:2 * CH[0]] = \
                w0[:, :, kh, kw].T
    return out


def _w2_passes(w1):
    """[128, 4*128]: per pass block, rows 0-63 = the tap read from the
    pairing buffer's low half, rows 64-127 = its high half. Blocks:
    (0,0)+(0,1), (1,0)+(0,2), (1,1)+(1,2), (2,0)+(2,1)."""
    a = w1.transpose(1, 2, 3, 0)  # [cin, kh, kw, cout]
    blocks = [((0, 0), (0, 1)), ((1, 0), (0, 2)),
              ((1, 1), (1, 2)), ((2, 0), (2, 1))]
    cols = [np.concatenate([a[:, lo[0], lo[1], :], a[:, hi[0], hi[1], :]],
                           axis=0) for lo, hi in blocks]
    return np.ascontiguousarray(np.concatenate(cols, axis=1))


def _prep_consts(w0, b0, w1, b1, w2, b2, w3, b3):
    wc = np.zeros((2 * CH[0], WCOLS), dtype=np.float32)
    wc[:, WOFF[0]:WOFF[0] + 4 * CH[1]] = _w2_passes(np.asarray(w1))
    # (2,2) tap weights duplicated on both PE row halves (row-tiled
    # concurrent pair: even banks use rows 0-63 vs copy A, odd banks
    # rows 64-127 vs shifted copy B)
    w22 = np.ascontiguousarray(
        np.asarray(w1).transpose(1, 2, 3, 0)[:, 2, 2, :])
    wc[0:CH[0], WOFF[1]:WOFF[1] + CH[1]] = w22
    wc[CH[0]:2 * CH[0], WOFF[1]:WOFF[1] + CH[1]] = w22
    wc[:, WOFF[2]:WOFF[2] + 9 * CH[2]] = np.asarray(w2).transpose(
        1, 2, 3, 0).reshape(CH[1], 9 * CH[2])
    wc[:, WOFF[3]:WOFF[3] + 9 * CH[3]] = np.asarray(w3).transpose(
        1, 2, 3, 0).reshape(CH[2], 9 * CH[3])
    fc = np.zeros((2 * CH[0], FCOLS), dtype=np.float32)
    fc[:, 0] = np.tile(np.asarray(b0, dtype=np.float32), 2)
    fc[:, 1] = np.asarray(b1, dtype=np.float32)
    fc[:, 2] = np.asarray(b2, dtype=np.float32)
    fc[:, 3] = np.asarray(b3, dtype=np.float32)
    fc[:, 4:4 + T] = _pe_table()
    return {"w1c": _cast_bf16(_w1_layout(np.asarray(w0))),
            "wc": _cast_bf16(wc), "fc": np.ascontiguousarray(fc)}


_prog_cache: dict[int, object] = {}


def _get_program(nimg: int):
    if nimg not in _prog_cache:
        _prog_cache[nimg] = _build(nimg)
    return _prog_cache[nimg]


_runner_cache: dict[int, object] = {}


def _get_runner(nimg: int):
    """A reusable jitted SPMD executor for the per-core program (avoids
    re-tracing/re-lowering on every kernel() call)."""
    if nimg in _runner_cache:
        return _runner_cache[nimg]

    import jax
    import jax.numpy as jnp
    from concourse.bass2jax import (
        install_neuronx_cc_hook, partition_id_tensor, _bass_exec_p)
    from jax.sharding import Mesh, PartitionSpec, NamedSharding
    from jax.experimental.shard_map import shard_map

    nc = _get_program(nimg)
    install_neuronx_cc_hook()

    partition_name = (nc.partition_id_tensor.name
                      if nc.partition_id_tensor else None)
    in_names, out_names, out_avals, zero_shapes = [], [], [], []
    for alloc in nc.m.functions[0].allocations:
        if not isinstance(alloc, mybir.MemoryLocationSet):
            continue
        name = alloc.memorylocations[0].name
        if alloc.kind == "ExternalInput":
            if name != partition_name:
                in_names.append(name)
        elif alloc.kind == "ExternalOutput":
            shape = tuple(alloc.tensor_shape)
            dtype = mybir.dt.np(alloc.dtype)
            out_names.append(name)
            out_avals.append(jax.core.ShapedArray(shape, dtype))
            zero_shapes.append((shape, dtype))
    n_params = len(in_names)
    n_outs = len(out_names)
    all_in_names = list(in_names) + list(out_names)
    if partition_name is not None:
        all_in_names.append(partition_name)

    def _body(*args):
        operands = list(args)
        if partition_name is not None:
            operands.append(partition_id_tensor())
        outs = _bass_exec_p.bind(
            *operands,
            out_avals=tuple(out_avals),
            in_names=tuple(all_in_names),
            out_names=tuple(out_names),
            lowering_input_output_aliases=(),
            sim_require_finite=True,
            sim_require_nnan=True,
            nc=nc,
        )
        return tuple(outs)

    devices = jax.devices()[:N_CORES]
    mesh = Mesh(np.asarray(devices), ("core",))
    sh = NamedSharding(mesh, PartitionSpec("core"))
    donate = tuple(range(n_params, n_params + n_outs))
    sharded = jax.jit(
        shard_map(_body, mesh=mesh,
                  in_specs=(PartitionSpec("core"),) * (n_params + n_outs),
                  out_specs=(PartitionSpec("core"),) * n_outs,
                  check_rep=False),
        donate_argnums=donate, keep_unused=True)
    zeros_fn = jax.jit(
        lambda: tuple(
            jnp.zeros((N_CORES * s[0], *s[1:]), d) for s, d in zero_shapes),
        out_shardings=(sh,) * n_outs)

    def run(in_maps):
        concat_in = [
            np.concatenate([np.asarray(in_maps[c][nm])
                            for c in range(N_CORES)], axis=0)
            for nm in in_names
        ]
        dev_in = [jax.device_put(a, sh) for a in concat_in]
        outs = sharded(*dev_in, *zeros_fn())
        oi = out_names.index("out")
        return np.asarray(outs[oi])

    _runner_cache[nimg] = run
    return run


def _stage_x(x_core):
    """[n, CIN, H, W] f32 -> host im2col fold: [n, K1, NPIX] bf16 where
    row kw*15 + kh*5 + c at column (r, col) holds the zero-padded input
    value x[c, r + kh - 1, col + kw - 1]; kw=3 is the extra tap column
    the odd pixel of each packed pair needs."""
    n = x_core.shape[0]
    xp = np.zeros((n, CIN, H + 2, W + 3), dtype=np.float32)
    xp[:, :, 1:1 + H, 1:1 + W] = x_core
    x9 = np.empty((n, K1, NPIX), dtype=np.float32)
    for kw in range(4):
        for kh in range(3):
            base = kw * 15 + kh * CIN
            x9[:, base:base + CIN] = xp[:, :, kh:kh + H, kw:kw + W].reshape(
                n, CIN, NPIX)
    return _cast_bf16(x9)


def make_in_maps(x, w0, b0, w1, b1, w2, b2, w3, b3):
    """Shard the full inputs into the 8 per-core input maps."""
    consts = _prep_consts(w0, b0, w1, b1, w2, b2, w3, b3)
    bpc = B // N_CORES  # batches per core
    in_maps = []
    for c in range(N_CORES):
        xs = _stage_x(
            np.asarray(x)[c * bpc:(c + 1) * bpc].reshape(
                bpc * T, CIN, H, W))
        in_maps.append({"xin": xs, **consts})
    return in_maps


def kernel(x, w0, b0, w1, b1, w2, b2, w3, b3):
    nimg = (B // N_CORES) * T
    run = _get_runner(nimg)
    in_maps = make_in_maps(x, w0, b0, w1, b1, w2, b2, w3, b3)
    glob = run(in_maps)  # [8*nimg, 128, 4096] bf16
    bpc = B // N_CORES
    out = glob.reshape(N_CORES * bpc, T, CH[3], H, W).reshape(
        B, T, CH[3], H, W)
    return np.ascontiguousarray(out.astype(np.float32))


# revision 21
# speedup vs baseline: 6.0054x; 1.2358x over previous
"""Trainium2 Bass kernel for nn_FeatureEmbedding (4-layer 3x3 conv CNN
with LeakyReLU + sinusoidal positional-encoding add).

Strategy
--------
Data-parallel over the batch dim: 32 batches x 12 frames = 384 images;
each of the 8 NeuronCores processes 48 images (4 batches). Per image
the whole layer chain runs out of SBUF.

PSUM is managed as 8 rotating SINGLE-bank [C, 512] tiles (one bank
each) with per-bank ScalarE drains. Profiling the double-bank variant
showed ~500 matmuls/run stalling ~0.4-1.1us each on bank-recycle
semaphores (drain latency of [128,1024] ACT reads); single-bank
granularity halves the drain latency and doubles the rotation slack,
keeping the PE stream within a few % of the pure-matmul floor
(measured device span 2.065ms vs the 1.925ms 96256-cycle/image floor).

  - Input: host-staged im2col fold x9 [60, 4096] bf16 -> ONE flat DMA
    per image, prefetched one image ahead on the SP queue. Startup
    order: w1 -> biases/pe -> x9[0] -> bulk weights, so layer 1
    starts ~3us earlier than with one packed constant DMA.
  - Layer 1 is 2-pixel-packed: stationary [60, 128] computes the even
    pixel's 64 channels in PE columns 0-63 and the odd pixel's in
    64-127; 4 N=512 matmuls/image (one per bank, 16 image rows each).
    Per-bank drains write interleaved even/odd columns; the odd-half
    drain reads PSUM partitions 64-127 into SBUF partitions 0-63.
  - Layers 2-4 are shift-GEMM over zero-padded [C, 66*66] activation
    buffers, one 512-pixel (8-row) column group per PSUM bank.
  - Layer 2 runs per bank-pair: the (2,2) K=64 tap for the even bank
    (weights on PE rows 0-63 vs h1 copy A) and the odd bank (weights
    on rows 64-127 vs the shifted copy B) are issued back-to-back as
    row-tiled matmuls that execute CONCURRENTLY in the PE array
    (tile_position row groups), halving the K=64 tap's cost. The 4
    paired K=128 taps then run against h1 = [A; A<<1] and h1x =
    [A<<64; A] pairing buffers (chunked SBUF-SBUF copies on the ACT
    queue, issued right behind the L1 drains each chunk depends on).
  - Layers 3-4 are bank-outer (9 taps accumulate back-to-back, then
    the bank drains while the next bank computes), which staggers the
    drains evenly instead of queueing 4 of them at each layer edge.
  - ScalarE drains every bank with fused Lrelu(psum + bias); layer 4
    drains to bf16 and the otherwise-idle DVE adds pe[:, t] per bank
    right behind each drain. The NEXT image's layer-1 banks are
    emitted behind every other freed L4 bank so the cross-image
    pipeline never waits out a drain phase. Output DMAs ride the SP
    queue; the final image streams out per bank to shorten the tail.
    The host casts bf16 -> f32.

Constants ship as 4 DMAs (w1, biases+pe, then the bulk bf16 weight
block) pre-marshaled on the host into the [K, M] stationary layouts
the PE wants. Two activation-buffer sets alternate between images so
DMA/PE/ACT pipeline across images.
"""

import numpy as np

import concourse.bass as bass
import concourse.bacc as bacc
import concourse.mybir as mybir
import concourse.tile as tile

F32 = mybir.dt.float32
BF16 = mybir.dt.bfloat16
AF = mybir.ActivationFunctionType

N_CORES = 8
B, T, CIN, H, W = 32, 12, 5, 64, 64
K1 = 60                # 12 tap rows x 5 cin: kw in 0..3 (2-px packing)
CH = [64, 128, 128, 128]
NPIX = H * W           # 4096
PITCH = W + 2          # 66 (padded row pitch for h buffers)
PAD = PITCH * PITCH    # 4356
NBANK = 8              # 512-pixel column groups (one PSUM bank each)
RPT = H // NBANK       # 8 rows per bank group
TILEPIX = RPT * W      # 512
ALPHA = 0.01           # LeakyReLU negative slope

TAPS = [(kh, kw) for kh in range(3) for kw in range(3)]

# packed-constant column offsets in wc: w2n | w22 | w3 | w4
WOFF = [0, 4 * CH[1], 5 * CH[1], 5 * CH[1] + 9 * CH[2]]
WCOLS = WOFF[3] + 9 * CH[3]
FCOLS = 4 + T

# L1 bank boundaries in padded flat space (4 banks x 16 rows)
CB = (0, PITCH * 17 - 1, PITCH * 33 - 1, PITCH * 49 - 1, PAD - 1)
SBL = (0, CB[1] - W, CB[2] - W, CB[3] - W, PAD - W)


def _build(nimg: int):
    """Build the per-core Bass program (SPMD: same program on all cores)."""
    nc = bacc.Bacc("TRN2", target_bir_lowering=False, debug=False)

    xin = nc.dram_tensor("xin", [nimg, K1, NPIX], BF16, kind="ExternalInput")
    w1d = nc.dram_tensor("w1c", [K1, 2 * CH[0]], BF16, kind="ExternalInput")
    wcd = nc.dram_tensor("wc", [2 * CH[0], WCOLS], BF16,
                         kind="ExternalInput")
    fcd = nc.dram_tensor("fc", [2 * CH[0], FCOLS], F32,
                         kind="ExternalInput")
    outd = nc.dram_tensor("out", [nimg, CH[3], NPIX], BF16,
                          kind="ExternalOutput")

    with tile.TileContext(nc) as tc:
        with (
            tc.tile_pool(name="wpool", bufs=1) as wp,
            tc.tile_pool(name="bpool", bufs=1) as bp,
            tc.tile_pool(name="psum", bufs=8, space="PSUM") as pp,
        ):
            # --- constants (SP queue order = need order) ---
            w1s = wp.tile([K1, 2 * CH[0]], BF16)
            nc.sync.dma_start(out=w1s, in_=w1d[:, :])
            fcs = wp.tile([2 * CH[0], FCOLS], F32)
            nc.sync.dma_start(out=fcs, in_=fcd[:, :])
            # x9 for image 0 goes next (sets built below), then bulk wc
            wcs = wp.tile([2 * CH[0], WCOLS], BF16)

            w2ns = wcs[:, WOFF[0]:WOFF[0] + 4 * CH[1]]
            w22s = wcs[:, WOFF[1]:WOFF[1] + CH[1]]
            w3s = wcs[:, WOFF[2]:WOFF[2] + 9 * CH[2]]
            w4s = wcs[:, WOFF[3]:WOFF[3] + 9 * CH[3]]
            b1s = fcs[:, 0:1]
            b2s = fcs[:, 1:2]
            b3s = fcs[:, 2:3]
            b4s = fcs[:, 3:4]
            pes = fcs[:, 4:4 + T]

            # --- persistent activation buffers, double-buffered ---
            sets = []
            for s in range(2):
                x9 = bp.tile([K1, NPIX], BF16, name=f"x9_{s}")
                h1 = bp.tile([2 * CH[0], PAD], BF16, name=f"h1_{s}")
                h1x = bp.tile([2 * CH[0], PAD], BF16, name=f"h1x_{s}")
                h2 = bp.tile([CH[1], PAD], BF16, name=f"h2_{s}")
                h3 = bp.tile([CH[2], PAD], BF16, name=f"h3_{s}")
                h4 = bp.tile([CH[3], NPIX], BF16, name=f"h4_{s}")
                # one-time zeroing of the halo ring
                for buf, np_ in ((h1, CH[0]), (h2, CH[1]), (h3, CH[2])):
                    v = buf.rearrange("p (r c) -> p r c", c=PITCH)
                    u = mybir.dt.uint16
                    nc.vector.memset(v[0:np_, 0:1, :].bitcast(u), 0.0)
                    nc.vector.memset(
                        v[0:np_, PITCH - 1:PITCH, :].bitcast(u), 0.0)
                    nc.vector.memset(
                        v[0:np_, 1:PITCH - 1, 0:1].bitcast(u), 0.0)
                    nc.vector.memset(
                        v[0:np_, 1:PITCH - 1, PITCH - 1:PITCH].bitcast(u),
                        0.0)
                sets.append((x9, h1, h1x, h2, h3, h4))

            # preload x9 for image 0, then the bulk weights
            nc.sync.dma_start(out=sets[0][0], in_=xin[0])
            nc.sync.dma_start(out=wcs, in_=wcd[:, :])

            def l1_bank(img, j):
                """2-px-packed layer-1 bank j (of 4): one [128, 512] PSUM
                bank covers 1024 pixels (16 image rows) -- stationary
                cols 0-63 are the even pixel's channels, 64-127 the odd
                pixel's. Copy-B / h1x pairing chunks (one double-size set
                per bank pair, 6 copies/image) are issued on the ACT
                queue right behind the drains they depend on."""
                x9 = sets[img % 2][0]
                xv = x9.rearrange("p (n two) -> p n two", two=2)
                ps = pp.tile([2 * CH[0], TILEPIX], F32,
                             name=f"ps1_{img}_{j}", tag="ps")
                c0 = j * TILEPIX
                nc.tensor.matmul(ps, w1s, xv[:, c0:c0 + TILEPIX, 0:1],
                                 start=True, stop=True)
                h1 = sets[img % 2][1]
                h1q = h1.rearrange("p (r ct two) -> p r ct two", ct=33,
                                   two=2)
                r0 = 1 + 16 * j
                nc.scalar.activation(
                    h1q[0:CH[0], r0:r0 + 16, 0:32, 1:2], ps[0:CH[0], :],
                    AF.Lrelu, bias=b1s[0:CH[0], 0:1], scale=1.0,
                    alpha=ALPHA)
                nc.scalar.activation(
                    h1q[0:CH[0], r0:r0 + 16, 1:33, 0:1],
                    ps[CH[0]:2 * CH[0], :], AF.Lrelu,
                    bias=b1s[CH[0]:2 * CH[0], 0:1], scale=1.0, alpha=ALPHA)
                if j in (1, 3):
                    h1x = sets[img % 2][2]
                    c0_, c1_ = CB[j - 1], CB[j + 1]
                    s0_, s1_ = SBL[j - 1], SBL[j + 1]
                    nc.scalar.dma_start(
                        out=h1[CH[0]:2 * CH[0], c0_:c1_],
                        in_=h1[0:CH[0], c0_ + 1:c1_ + 1])
                    nc.scalar.dma_start(
                        out=h1x[0:CH[0], s0_:s1_],
                        in_=h1[0:CH[0], s0_ + W:s1_ + W])
                    nc.scalar.dma_start(
                        out=h1x[CH[0]:2 * CH[0], c0_:c1_],
                        in_=h1[0:CH[0], c0_:c1_])

            for j in range(4):
                l1_bank(0, j)

            for img in range(nimg):
                x9, h1, h1x, h2, h3, h4 = sets[img % 2]
                t = img % T
                h1v = h1.rearrange("p (r c) -> p r c", c=PITCH)
                h1xv = h1x.rearrange("p (r c) -> p r c", c=PITCH)
                h2v = h2.rearrange("p (r c) -> p r c", c=PITCH)
                h3v = h3.rearrange("p (r c) -> p r c", c=PITCH)

                # prefetch next image's x9
                if img + 1 < nimg:
                    nc.sync.dma_start(out=sets[(img + 1) % 2][0],
                                      in_=xin[img + 1])

                # layer 2 per bank-pair: 3 paired K=128 passes, then the
                # (2,2) K=64 taps of both banks back-to-back as row-tiled
                # matmuls that run CONCURRENTLY in the PE (copy A on PE
                # rows 0-63, shifted copy B on rows 64-127), then the h1x
                # pass last (it waits on the latest pairing copies).
                a128 = slice(0, 2 * CH[0])
                for jp in range(4):
                    pa = pp.tile([CH[1], TILEPIX], F32,
                                 name=f"ps2_{img}_{2 * jp}", tag="ps")
                    pb = pp.tile([CH[1], TILEPIX], F32,
                                 name=f"ps2_{img}_{2 * jp + 1}", tag="ps")
                    ra = 2 * jp * RPT
                    rb = ra + RPT
                    for wap, view, kh0, c0, first in (
                        (w2ns[:, 0:CH[1]], h1v, 0, 0, True),
                        (w2ns[:, 2 * CH[1]:3 * CH[1]], h1v, 1, 1, False),
                        (w2ns[:, 3 * CH[1]:4 * CH[1]], h1v, 2, 0, False),
                    ):
                        for ps_, r0 in ((pa, ra), (pb, rb)):
                            nc.tensor.matmul(
                                ps_, wap,
                                view[a128, r0 + kh0:r0 + kh0 + RPT,
                                     c0:c0 + W],
                                start=first, stop=False)
                    # (2,2) K=64 concurrent row-tiled pair (copy-B dep)
                    nc.tensor.matmul(
                        pa, w22s[0:CH[0], :],
                        h1v[0:CH[0], ra + 2:ra + 2 + RPT, 2:2 + W],
                        start=False, stop=False)
                    nc.tensor.matmul(
                        pb, w22s[CH[0]:2 * CH[0], :],
                        h1v[CH[0]:2 * CH[0], rb + 2:rb + 2 + RPT, 1:1 + W],
                        start=False, stop=False)
                    for ps_, r0 in ((pa, ra), (pb, rb)):
                        nc.tensor.matmul(
                            ps_, w2ns[:, CH[1]:2 * CH[1]],
                            h1xv[a128, r0:r0 + RPT, 2:2 + W],
                            start=False, stop=True)
                    for ps_, r0 in ((pa, ra), (pb, rb)):
                        nc.scalar.activation(
                            h2v[:, 1 + r0:1 + r0 + RPT, 1:1 + W], ps_,
                            AF.Lrelu, bias=b2s[:, 0:1], scale=1.0,
                            alpha=ALPHA)

                # layers 3 and 4: bank-outer, drains staggered behind
                # each bank's 9-tap accumulation
                for g in range(NBANK):
                    ps3 = pp.tile([CH[2], TILEPIX], F32,
                                  name=f"ps3_{img}_{g}", tag="ps")
                    r0 = g * RPT
                    for ti, (kh, kw) in enumerate(TAPS):
                        tap = kh * 3 + kw
                        nc.tensor.matmul(
                            ps3, w3s[:, tap * CH[2]:(tap + 1) * CH[2]],
                            h2v[:, r0 + kh:r0 + kh + RPT, kw:kw + W],
                            start=(ti == 0), stop=(ti == 8))
                    nc.scalar.activation(
                        h3v[:, 1 + r0:1 + r0 + RPT, 1:1 + W], ps3,
                        AF.Lrelu, bias=b3s[:, 0:1], scale=1.0, alpha=ALPHA)

                for g in range(NBANK):
                    ps4 = pp.tile([CH[3], TILEPIX], F32,
                                  name=f"ps4_{img}_{g}", tag="ps")
                    r0 = g * RPT
                    for ti, (kh, kw) in enumerate(TAPS):
                        tap = kh * 3 + kw
                        nc.tensor.matmul(
                            ps4, w4s[:, tap * CH[3]:(tap + 1) * CH[3]],
                            h3v[:, r0 + kh:r0 + kh + RPT, kw:kw + W],
                            start=(ti == 0), stop=(ti == 8))
                    seg = h4[:, g * TILEPIX:(g + 1) * TILEPIX]
                    nc.scalar.activation(
                        seg, ps4, AF.Lrelu, bias=b4s[:, 0:1], scale=1.0,
                        alpha=ALPHA)
                    nc.vector.tensor_scalar_add(seg, seg, pes[:, t:t + 1])
                    if img + 1 < nimg and g < 4:
                        l1_bank(img + 1, g)
                    if img + 1 >= nimg:
                        nc.sync.dma_start(
                            out=outd[img, :, g * TILEPIX:(g + 1) * TILEPIX],
                            in_=seg)

                # store (SP queue; the final image streamed out per bank
                # above so the program tail is one chunk)
                if img + 1 < nimg:
                    nc.sync.dma_start(out=outd[img], in_=h4)

    nc.compile()
    return nc


def _pe_table():
    d = np.arange(CH[3])
    d_even = (d // 2) * 2
    tt = np.arange(T, dtype=np.float64)
    arg = tt[:, None] / np.power(10000.0, d_even / CH[3])
    pe = np.where(d % 2 == 0, np.sin(arg), np.cos(arg))  # [T, D]
    return np.ascontiguousarray(pe.T.astype(np.float32))  # [D, T]


def _cast_bf16(a):
    """fp32 -> bf16 (round-to-nearest-even) on the host so on-chip DMAs
    are plain copies."""
    import ml_dtypes
    return np.ascontiguousarray(np.asarray(a, dtype=np.float32)).astype(
        ml_dtypes.bfloat16)


def _w1_layout(w0):
    """[K1, 128] 2-px stationary: cols 0-63 = even pixel's channels
    (tap rows kw 0-2), cols 64-127 = odd pixel's (tap rows kw 1-3 hold
    its kw-1 weights)."""
    out = np.zeros((K1, 2 * CH[0]), dtype=np.float32)
    for kw in range(3):
        for kh in range(3):
            base = kw * 15 + kh * CIN
            out[base:base + CIN, 0:CH[0]] = w0[:, :, kh, kw].T
            out[base + 15:base + 15 + CIN, CH[0]:2 * CH[0]] = \
                w0[:, :, kh, kw].T
    return out


def _w2_passes(w1):
    """[128, 4*128]: per pass block, rows 0-63 = the tap read from the
    pairing buffer's low half, rows 64-127 = its high half. Blocks:
    (0,0)+(0,1), (1,0)+(0,2), (1,1)+(1,2), (2,0)+(2,1)."""
    a = w1.transpose(1, 2, 3, 0)  # [cin, kh, kw, cout]
    blocks = [((0, 0), (0, 1)), ((1, 0), (0, 2)),
              ((1, 1), (1, 2)), ((2, 0), (2, 1))]
    cols = [np.concatenate([a[:, lo[0], lo[1], :], a[:, hi[0], hi[1], :]],
                           axis=0) for lo, hi in blocks]
    return np.ascontiguousarray(np.concatenate(cols, axis=1))


def _prep_consts(w0, b0, w1, b1, w2, b2, w3, b3):
    wc = np.zeros((2 * CH[0], WCOLS), dtype=np.float32)
    wc[:, WOFF[0]:WOFF[0] + 4 * CH[1]] = _w2_passes(np.asarray(w1))
    # (2,2) tap weights duplicated on both PE row halves (row-tiled
    # concurrent pair: even banks use rows 0-63 vs copy A, odd banks
    # rows 64-127 vs shifted copy B)
    w22 = np.ascontiguousarray(
        np.asarray(w1).transpose(1, 2, 3, 0)[:, 2, 2, :])
    wc[0:CH[0], WOFF[1]:WOFF[1] + CH[1]] = w22
    wc[CH[0]:2 * CH[0], WOFF[1]:WOFF[1] + CH[1]] = w22
    wc[:, WOFF[2]:WOFF[2] + 9 * CH[2]] = np.asarray(w2).transpose(
        1, 2, 3, 0).reshape(CH[1], 9 * CH[2])
    wc[:, WOFF[3]:WOFF[3] + 9 * CH[3]] = np.asarray(w3).transpose(
        1, 2, 3, 0).reshape(CH[2], 9 * CH[3])
    fc = np.zeros((2 * CH[0], FCOLS), dtype=np.float32)
    fc[:, 0] = np.tile(np.asarray(b0, dtype=np.float32), 2)
    fc[:, 1] = np.asarray(b1, dtype=np.float32)
    fc[:, 2] = np.asarray(b2, dtype=np.float32)
    fc[:, 3] = np.asarray(b3, dtype=np.float32)
    fc[:, 4:4 + T] = _pe_table()
    return {"w1c": _cast_bf16(_w1_layout(np.asarray(w0))),
            "wc": _cast_bf16(wc), "fc": np.ascontiguousarray(fc)}


_prog_cache: dict[int, object] = {}


def _get_program(nimg: int):
    if nimg not in _prog_cache:
        _prog_cache[nimg] = _build(nimg)
    return _prog_cache[nimg]


_runner_cache: dict[int, object] = {}


def _get_runner(nimg: int):
    """A reusable jitted SPMD executor for the per-core program (avoids
    re-tracing/re-lowering on every kernel() call)."""
    if nimg in _runner_cache:
        return _runner_cache[nimg]

    import jax
    import jax.numpy as jnp
    from concourse.bass2jax import (
        install_neuronx_cc_hook, partition_id_tensor, _bass_exec_p)
    from jax.sharding import Mesh, PartitionSpec, NamedSharding
    from jax.experimental.shard_map import shard_map

    nc = _get_program(nimg)
    install_neuronx_cc_hook()

    partition_name = (nc.partition_id_tensor.name
                      if nc.partition_id_tensor else None)
    in_names, out_names, out_avals, zero_shapes = [], [], [], []
    for alloc in nc.m.functions[0].allocations:
        if not isinstance(alloc, mybir.MemoryLocationSet):
            continue
        name = alloc.memorylocations[0].name
        if alloc.kind == "ExternalInput":
            if name != partition_name:
                in_names.append(name)
        elif alloc.kind == "ExternalOutput":
            shape = tuple(alloc.tensor_shape)
            dtype = mybir.dt.np(alloc.dtype)
            out_names.append(name)
            out_avals.append(jax.core.ShapedArray(shape, dtype))
            zero_shapes.append((shape, dtype))
    n_params = len(in_names)
    n_outs = len(out_names)
    all_in_names = list(in_names) + list(out_names)
    if partition_name is not None:
        all_in_names.append(partition_name)

    def _body(*args):
        operands = list(args)
        if partition_name is not None:
            operands.append(partition_id_tensor())
        outs = _bass_exec_p.bind(
            *operands,
            out_avals=tuple(out_avals),
            in_names=tuple(all_in_names),
            out_names=tuple(out_names),
            lowering_input_output_aliases=(),
            sim_require_finite=True,
            sim_require_nnan=True,
            nc=nc,
        )
        return tuple(outs)

    devices = jax.devices()[:N_CORES]
    mesh = Mesh(np.asarray(devices), ("core",))
    sh = NamedSharding(mesh, PartitionSpec("core"))
    donate = tuple(range(n_params, n_params + n_outs))
    sharded = jax.jit(
        shard_map(_body, mesh=mesh,
                  in_specs=(PartitionSpec("core"),) * (n_params + n_outs),
                  out_specs=(PartitionSpec("core"),) * n_outs,
                  check_rep=False),
        donate_argnums=donate, keep_unused=True)
    zeros_fn = jax.jit(
        lambda: tuple(
            jnp.zeros((N_CORES * s[0], *s[1:]), d) for s, d in zero_shapes),
        out_shardings=(sh,) * n_outs)

    def run(in_maps):
        concat_in = [
            np.concatenate([np.asarray(in_maps[c][nm])
                            for c in range(N_CORES)], axis=0)
            for nm in in_names
        ]
        dev_in = [jax.device_put(a, sh) for a in concat_in]
        outs = sharded(*dev_in, *zeros_fn())
        oi = out_names.index("out")
        return np.asarray(outs[oi])

    _runner_cache[nimg] = run
    return run


def _stage_x(x_core):
    """[n, CIN, H, W] f32 -> host im2col fold: [n, K1, NPIX] bf16 where
    row kw*15 + kh*5 + c at column (r, col) holds the zero-padded input
    value x[c, r + kh - 1, col + kw - 1]; kw=3 is the extra tap column
    the odd pixel of each packed pair needs."""
    n = x_core.shape[0]
    xp = np.zeros((n, CIN, H + 2, W + 3), dtype=np.float32)
    xp[:, :, 1:1 + H, 1:1 + W] = x_core
    x9 = np.empty((n, K1, NPIX), dtype=np.float32)
    for kw in range(4):
        for kh in range(3):
            base = kw * 15 + kh * CIN
            x9[:, base:base + CIN] = xp[:, :, kh:kh + H, kw:kw + W].reshape(
                n, CIN, NPIX)
    return _cast_bf16(x9)


def make_in_maps(x, w0, b0, w1, b1, w2, b2, w3, b3):
    """Shard the full inputs into the 8 per-core input maps."""
    consts = _prep_consts(w0, b0, w1, b1, w2, b2, w3, b3)
    bpc = B // N_CORES  # batches per core
    in_maps = []
    for c in range(N_CORES):
        xs = _stage_x(
            np.asarray(x)[c * bpc:(c + 1) * bpc].reshape(
                bpc * T, CIN, H, W))
        in_maps.append({"xin": xs, **consts})
    return in_maps


def kernel(x, w0, b0, w1, b1, w2, b2, w3, b3):
    nimg = (B // N_CORES) * T
    run = _get_runner(nimg)
    in_maps = make_in_maps(x, w0, b0, w1, b1, w2, b2, w3, b3)
    glob = run(in_maps)  # [8*nimg, 128, 4096] bf16
    bpc = B // N_CORES
    out = glob.reshape(N_CORES * bpc, T, CH[3], H, W).reshape(
        B, T, CH[3], H, W)
    return np.ascontiguousarray(out.astype(np.float32))


# revision 23
# speedup vs baseline: 6.1565x; 1.0252x over previous
"""Trainium2 Bass kernel for nn_FeatureEmbedding (4-layer 3x3 conv CNN
with LeakyReLU + sinusoidal positional-encoding add).

Strategy
--------
Data-parallel over the batch dim: 32 batches x 12 frames = 384 images;
each of the 8 NeuronCores processes 48 images (4 batches). Per image
the whole layer chain runs out of SBUF.

PSUM is managed as 8 rotating SINGLE-bank [C, 512] tiles (one bank
each) with per-bank ScalarE drains. Profiling the double-bank variant
showed ~500 matmuls/run stalling ~0.4-1.1us each on bank-recycle
semaphores (drain latency of [128,1024] ACT reads); single-bank
granularity halves the drain latency and doubles the rotation slack,
keeping the PE stream within a few % of the pure-matmul floor
(measured device span 2.065ms vs the 1.925ms 96256-cycle/image floor).

  - Input: host-staged im2col fold x9 [60, 4096] bf16 -> ONE flat DMA
    per image, prefetched one image ahead on the SP queue. Startup
    order: w1 -> biases/pe -> x9[0] -> bulk weights, so layer 1
    starts ~3us earlier than with one packed constant DMA.
  - Layer 1 is 2-pixel-packed: stationary [60, 128] computes the even
    pixel's 64 channels in PE columns 0-63 and the odd pixel's in
    64-127; 4 N=512 matmuls/image (one per bank, 16 image rows each).
    Per-bank drains write interleaved even/odd columns; the odd-half
    drain reads PSUM partitions 64-127 into SBUF partitions 0-63.
  - Layers 2-4 are shift-GEMM over zero-padded [C, 66*66] activation
    buffers, one 512-pixel (8-row) column group per PSUM bank.
  - Layer 2 runs per bank-pair: the (2,2) K=64 tap for the even bank
    (weights on PE rows 0-63 vs h1 copy A) and the odd bank (weights
    on rows 64-127 vs the shifted copy B) are issued back-to-back as
    row-tiled matmuls that execute CONCURRENTLY in the PE array
    (tile_position row groups), halving the K=64 tap's cost. The 4
    paired K=128 taps then run against h1 = [A; A<<1] and h1x =
    [A<<64; A] pairing buffers (chunked SBUF-SBUF copies on the SP
    queue, issued right behind the L1 drains each chunk depends on).
  - Layers 3-4 are bank-outer (9 taps accumulate back-to-back, then
    the bank drains while the next bank computes), which staggers the
    drains evenly instead of queueing 4 of them at each layer edge.
  - ScalarE drains every bank with fused Lrelu(psum + bias); layer 4
    drains to bf16 and the otherwise-idle DVE adds pe[:, t] per bank
    right behind each drain. The NEXT image's layer-1 banks are
    emitted behind every other freed L4 bank so the cross-image
    pipeline never waits out a drain phase. Output DMAs ride the SP
    queue; the final image streams out per bank to shorten the tail.
    The host casts bf16 -> f32.

Constants ship as 4 DMAs (w1, biases+pe, then the bulk bf16 weight
block) pre-marshaled on the host into the [K, M] stationary layouts
the PE wants. Two activation-buffer sets alternate between images so
DMA/PE/ACT pipeline across images.
"""

import numpy as np

import concourse.bass as bass
import concourse.bacc as bacc
import concourse.mybir as mybir
import concourse.tile as tile

F32 = mybir.dt.float32
BF16 = mybir.dt.bfloat16
AF = mybir.ActivationFunctionType

N_CORES = 8
B, T, CIN, H, W = 32, 12, 5, 64, 64
K1 = 60                # 12 tap rows x 5 cin: kw in 0..3 (2-px packing)
CH = [64, 128, 128, 128]
NPIX = H * W           # 4096
PITCH = W + 2          # 66 (padded row pitch for h buffers)
PAD = PITCH * PITCH    # 4356
NBANK = 8              # 512-pixel column groups (one PSUM bank each)
RPT = H // NBANK       # 8 rows per bank group
TILEPIX = RPT * W      # 512
ALPHA = 0.01           # LeakyReLU negative slope

TAPS = [(kh, kw) for kh in range(3) for kw in range(3)]

# packed-constant column offsets in wc: w2n | w22 | w3 | w4
WOFF = [0, 4 * CH[1], 5 * CH[1], 5 * CH[1] + 9 * CH[2]]
WCOLS = WOFF[3] + 9 * CH[3]
FCOLS = 4 + T

# L1 bank boundaries in padded flat space (4 banks x 16 rows)
CB = (0, PITCH * 17 - 1, PITCH * 33 - 1, PITCH * 49 - 1, PAD - 1)
SBL = (0, CB[1] - W, CB[2] - W, CB[3] - W, PAD - W)


def _build(nimg: int):
    """Build the per-core Bass program (SPMD: same program on all cores)."""
    nc = bacc.Bacc("TRN2", target_bir_lowering=False, debug=False)

    xin = nc.dram_tensor("xin", [nimg, K1, NPIX], BF16, kind="ExternalInput")
    w1d = nc.dram_tensor("w1c", [K1, 2 * CH[0]], BF16, kind="ExternalInput")
    wcd = nc.dram_tensor("wc", [2 * CH[0], WCOLS], BF16,
                         kind="ExternalInput")
    fcd = nc.dram_tensor("fc", [2 * CH[0], FCOLS], F32,
                         kind="ExternalInput")
    outd = nc.dram_tensor("out", [nimg, CH[3], NPIX], BF16,
                          kind="ExternalOutput")

    with tile.TileContext(nc) as tc:
        with (
            tc.tile_pool(name="wpool", bufs=1) as wp,
            tc.tile_pool(name="bpool", bufs=1) as bp,
            tc.tile_pool(name="psum", bufs=8, space="PSUM") as pp,
        ):
            # --- constants (SP queue order = need order) ---
            w1s = wp.tile([K1, 2 * CH[0]], BF16)
            nc.sync.dma_start(out=w1s, in_=w1d[:, :])
            fcs = wp.tile([2 * CH[0], FCOLS], F32)
            nc.sync.dma_start(out=fcs, in_=fcd[:, :])
            # x9 for image 0 goes next (sets built below), then bulk wc
            wcs = wp.tile([2 * CH[0], WCOLS], BF16)

            w2ns = wcs[:, WOFF[0]:WOFF[0] + 4 * CH[1]]
            w22s = wcs[:, WOFF[1]:WOFF[1] + CH[1]]
            w3s = wcs[:, WOFF[2]:WOFF[2] + 9 * CH[2]]
            w4s = wcs[:, WOFF[3]:WOFF[3] + 9 * CH[3]]
            b1s = fcs[:, 0:1]
            b2s = fcs[:, 1:2]
            b3s = fcs[:, 2:3]
            b4s = fcs[:, 3:4]
            pes = fcs[:, 4:4 + T]

            # --- persistent activation buffers, double-buffered ---
            sets = []
            for s in range(2):
                x9 = bp.tile([K1, NPIX], BF16, name=f"x9_{s}")
                h1 = bp.tile([2 * CH[0], PAD], BF16, name=f"h1_{s}")
                h1x = bp.tile([2 * CH[0], PAD], BF16, name=f"h1x_{s}")
                h2 = bp.tile([CH[1], PAD], BF16, name=f"h2_{s}")
                h3 = bp.tile([CH[2], PAD], BF16, name=f"h3_{s}")
                h4 = bp.tile([CH[3], NPIX], BF16, name=f"h4_{s}")
                # one-time zeroing of the halo ring
                for buf, np_ in ((h1, CH[0]), (h2, CH[1]), (h3, CH[2])):
                    v = buf.rearrange("p (r c) -> p r c", c=PITCH)
                    u = mybir.dt.uint16
                    nc.vector.memset(v[0:np_, 0:1, :].bitcast(u), 0.0)
                    nc.vector.memset(
                        v[0:np_, PITCH - 1:PITCH, :].bitcast(u), 0.0)
                    nc.vector.memset(
                        v[0:np_, 1:PITCH - 1, 0:1].bitcast(u), 0.0)
                    nc.vector.memset(
                        v[0:np_, 1:PITCH - 1, PITCH - 1:PITCH].bitcast(u),
                        0.0)
                sets.append((x9, h1, h1x, h2, h3, h4))

            # preload x9 for image 0, then the bulk weights
            nc.sync.dma_start(out=sets[0][0], in_=xin[0])
            nc.sync.dma_start(out=wcs, in_=wcd[:, :])

            def l1_bank(img, j):
                """2-px-packed layer-1 bank j (of 4): one [128, 512] PSUM
                bank covers 1024 pixels (16 image rows) -- stationary
                cols 0-63 are the even pixel's channels, 64-127 the odd
                pixel's. Copy-B / h1x pairing chunks (one double-size set
                per bank pair, 6 copies/image) ride the SP queue so the
                ACT sequencer carries ONLY drains (a HWDGE config costs
                ~667ns and was delaying L4 drains, and with them the
                PSUM bank recycle the PE waits on)."""
                x9 = sets[img % 2][0]
                xv = x9.rearrange("p (n two) -> p n two", two=2)
                ps = pp.tile([2 * CH[0], TILEPIX], F32,
                             name=f"ps1_{img}_{j}", tag="ps")
                c0 = j * TILEPIX
                nc.tensor.matmul(ps, w1s, xv[:, c0:c0 + TILEPIX, 0:1],
                                 start=True, stop=True)
                h1 = sets[img % 2][1]
                h1q = h1.rearrange("p (r ct two) -> p r ct two", ct=33,
                                   two=2)
                r0 = 1 + 16 * j
                nc.scalar.activation(
                    h1q[0:CH[0], r0:r0 + 16, 0:32, 1:2], ps[0:CH[0], :],
                    AF.Lrelu, bias=b1s[0:CH[0], 0:1], scale=1.0,
                    alpha=ALPHA)
                nc.scalar.activation(
                    h1q[0:CH[0], r0:r0 + 16, 1:33, 0:1],
                    ps[CH[0]:2 * CH[0], :], AF.Lrelu,
                    bias=b1s[CH[0]:2 * CH[0], 0:1], scale=1.0, alpha=ALPHA)
                if j in (1, 3):
                    h1x = sets[img % 2][2]
                    c0_, c1_ = CB[j - 1], CB[j + 1]
                    s0_, s1_ = SBL[j - 1], SBL[j + 1]
                    nc.sync.dma_start(
                        out=h1[CH[0]:2 * CH[0], c0_:c1_],
                        in_=h1[0:CH[0], c0_ + 1:c1_ + 1])
                    nc.sync.dma_start(
                        out=h1x[0:CH[0], s0_:s1_],
                        in_=h1[0:CH[0], s0_ + W:s1_ + W])
                    nc.sync.dma_start(
                        out=h1x[CH[0]:2 * CH[0], c0_:c1_],
                        in_=h1[0:CH[0], c0_:c1_])

            for j in range(4):
                l1_bank(0, j)

            for img in range(nimg):
                x9, h1, h1x, h2, h3, h4 = sets[img % 2]
                t = img % T
                h1v = h1.rearrange("p (r c) -> p r c", c=PITCH)
                h1xv = h1x.rearrange("p (r c) -> p r c", c=PITCH)
                h2v = h2.rearrange("p (r c) -> p r c", c=PITCH)
                h3v = h3.rearrange("p (r c) -> p r c", c=PITCH)

                # prefetch next image's x9
                if img + 1 < nimg:
                    nc.sync.dma_start(out=sets[(img + 1) % 2][0],
                                      in_=xin[img + 1])

                # layer 2 per bank-pair: 3 paired K=128 passes, then the
                # (2,2) K=64 taps of both banks back-to-back as row-tiled
                # matmuls that run CONCURRENTLY in the PE (copy A on PE
                # rows 0-63, shifted copy B on rows 64-127), then the h1x
                # pass last (it waits on the latest pairing copies).
                a128 = slice(0, 2 * CH[0])
                for jp in range(4):
                    pa = pp.tile([CH[1], TILEPIX], F32,
                                 name=f"ps2_{img}_{2 * jp}", tag="ps")
                    pb = pp.tile([CH[1], TILEPIX], F32,
                                 name=f"ps2_{img}_{2 * jp + 1}", tag="ps")
                    ra = 2 * jp * RPT
                    rb = ra + RPT
                    for wap, view, kh0, c0, first in (
                        (w2ns[:, 0:CH[1]], h1v, 0, 0, True),
                        (w2ns[:, 2 * CH[1]:3 * CH[1]], h1v, 1, 1, False),
                        (w2ns[:, 3 * CH[1]:4 * CH[1]], h1v, 2, 0, False),
                    ):
                        for ps_, r0 in ((pa, ra), (pb, rb)):
                            nc.tensor.matmul(
                                ps_, wap,
                                view[a128, r0 + kh0:r0 + kh0 + RPT,
                                     c0:c0 + W],
                                start=first, stop=False)
                    # (2,2) K=64 concurrent row-tiled pair (copy-B dep)
                    nc.tensor.matmul(
                        pa, w22s[0:CH[0], :],
                        h1v[0:CH[0], ra + 2:ra + 2 + RPT, 2:2 + W],
                        start=False, stop=False)
                    nc.tensor.matmul(
                        pb, w22s[CH[0]:2 * CH[0], :],
                        h1v[CH[0]:2 * CH[0], rb + 2:rb + 2 + RPT, 1:1 + W],
                        start=False, stop=False)
                    for ps_, r0 in ((pa, ra), (pb, rb)):
                        nc.tensor.matmul(
                            ps_, w2ns[:, CH[1]:2 * CH[1]],
                            h1xv[a128, r0:r0 + RPT, 2:2 + W],
                            start=False, stop=True)
                    for ps_, r0 in ((pa, ra), (pb, rb)):
                        nc.scalar.activation(
                            h2v[:, 1 + r0:1 + r0 + RPT, 1:1 + W], ps_,
                            AF.Lrelu, bias=b2s[:, 0:1], scale=1.0,
                            alpha=ALPHA)

                # layers 3 and 4: bank-outer, drains staggered behind
                # each bank's 9-tap accumulation
                for g in range(NBANK):
                    ps3 = pp.tile([CH[2], TILEPIX], F32,
                                  name=f"ps3_{img}_{g}", tag="ps")
                    r0 = g * RPT
                    for ti, (kh, kw) in enumerate(TAPS):
                        tap = kh * 3 + kw
                        nc.tensor.matmul(
                            ps3, w3s[:, tap * CH[2]:(tap + 1) * CH[2]],
                            h2v[:, r0 + kh:r0 + kh + RPT, kw:kw + W],
                            start=(ti == 0), stop=(ti == 8))
                    nc.scalar.activation(
                        h3v[:, 1 + r0:1 + r0 + RPT, 1:1 + W], ps3,
                        AF.Lrelu, bias=b3s[:, 0:1], scale=1.0, alpha=ALPHA)

                for g in range(NBANK):
                    ps4 = pp.tile([CH[3], TILEPIX], F32,
                                  name=f"ps4_{img}_{g}", tag="ps")
                    r0 = g * RPT
                    for ti, (kh, kw) in enumerate(TAPS):
                        tap = kh * 3 + kw
                        nc.tensor.matmul(
                            ps4, w4s[:, tap * CH[3]:(tap + 1) * CH[3]],
                            h3v[:, r0 + kh:r0 + kh + RPT, kw:kw + W],
                            start=(ti == 0), stop=(ti == 8))
                    seg = h4[:, g * TILEPIX:(g + 1) * TILEPIX]
                    nc.scalar.activation(
                        seg, ps4, AF.Lrelu, bias=b4s[:, 0:1], scale=1.0,
                        alpha=ALPHA)
                    nc.vector.tensor_scalar_add(seg, seg, pes[:, t:t + 1])
                    if img + 1 < nimg and g < 4:
                        l1_bank(img + 1, g)
                    if img + 1 >= nimg:
                        nc.sync.dma_start(
                            out=outd[img, :, g * TILEPIX:(g + 1) * TILEPIX],
                            in_=seg)

                # store (SP queue; the final image streamed out per bank
                # above so the program tail is one chunk)
                if img + 1 < nimg:
                    nc.sync.dma_start(out=outd[img], in_=h4)

    nc.compile()
    return nc


def _pe_table():
    d = np.arange(CH[3])
    d_even = (d // 2) * 2
    tt = np.arange(T, dtype=np.float64)
    arg = tt[:, None] / np.power(10000.0, d_even / CH[3])
    pe = np.where(d % 2 == 0, np.sin(arg), np.cos(arg))  # [T, D]
    return np.ascontiguousarray(pe.T.astype(np.float32))  # [D, T]


def _cast_bf16(a):
    """fp32 -> bf16 (round-to-nearest-even) on the host so on-chip DMAs
    are plain copies."""
    import ml_dtypes
    return np.ascontiguousarray(np.asarray(a, dtype=np.float32)).astype(
        ml_dtypes.bfloat16)


def _w1_layout(w0):
    """[K1, 128] 2-px stationary: cols 0-63 = even pixel's channels
    (tap rows kw 0-2), cols 64-127 = odd pixel's (tap rows kw 1-3 hold
    its kw-1 weights)."""
    out = np.zeros((K1, 2 * CH[0]), dtype=np.float32)
    for kw in range(3):
        for kh in range(3):
            base = kw * 15 + kh * CIN
            out[base:base + CIN, 0:CH[0]] = w0[:, :, kh, kw].T
            out[base + 15:base + 15 + CIN, CH[0]:2 * CH[0]] = \
                w0[:, :, kh, kw].T
    return out


def _w2_passes(w1):
    """[128, 4*128]: per pass block, rows 0-63 = the tap read from the
    pairing buffer's low half, rows 64-127 = its high half. Blocks:
    (0,0)+(0,1), (1,0)+(0,2), (1,1)+(1,2), (2,0)+(2,1)."""
    a = w1.transpose(1, 2, 3, 0)  # [cin, kh, kw, cout]
    blocks = [((0, 0), (0, 1)), ((1, 0), (0, 2)),
              ((1, 1), (1, 2)), ((2, 0), (2, 1))]
    cols = [np.concatenate([a[:, lo[0], lo[1], :], a[:, hi[0], hi[1], :]],
                           axis=0) for lo, hi in blocks]
    return np.ascontiguousarray(np.concatenate(cols, axis=1))


def _prep_consts(w0, b0, w1, b1, w2, b2, w3, b3):
    wc = np.zeros((2 * CH[0], WCOLS), dtype=np.float32)
    wc[:, WOFF[0]:WOFF[0] + 4 * CH[1]] = _w2_passes(np.asarray(w1))
    # (2,2) tap weights duplicated on both PE row halves (row-tiled
    # concurrent pair: even banks use rows 0-63 vs copy A, odd banks
    # rows 64-127 vs shifted copy B)
    w22 = np.ascontiguousarray(
        np.asarray(w1).transpose(1, 2, 3, 0)[:, 2, 2, :])
    wc[0:CH[0], WOFF[1]:WOFF[1] + CH[1]] = w22
    wc[CH[0]:2 * CH[0], WOFF[1]:WOFF[1] + CH[1]] = w22
    wc[:, WOFF[2]:WOFF[2] + 9 * CH[2]] = np.asarray(w2).transpose(
        1, 2, 3, 0).reshape(CH[1], 9 * CH[2])
    wc[:, WOFF[3]:WOFF[3] + 9 * CH[3]] = np.asarray(w3).transpose(
        1, 2, 3, 0).reshape(CH[2], 9 * CH[3])
    fc = np.zeros((2 * CH[0], FCOLS), dtype=np.float32)
    fc[:, 0] = np.tile(np.asarray(b0, dtype=np.float32), 2)
    fc[:, 1] = np.asarray(b1, dtype=np.float32)
    fc[:, 2] = np.asarray(b2, dtype=np.float32)
    fc[:, 3] = np.asarray(b3, dtype=np.float32)
    fc[:, 4:4 + T] = _pe_table()
    return {"w1c": _cast_bf16(_w1_layout(np.asarray(w0))),
            "wc": _cast_bf16(wc), "fc": np.ascontiguousarray(fc)}


_prog_cache: dict[int, object] = {}


def _get_program(nimg: int):
    if nimg not in _prog_cache:
        _prog_cache[nimg] = _build(nimg)
    return _prog_cache[nimg]


_runner_cache: dict[int, object] = {}


def _get_runner(nimg: int):
    """A reusable jitted SPMD executor for the per-core program (avoids
    re-tracing/re-lowering on every kernel() call)."""
    if nimg in _runner_cache:
        return _runner_cache[nimg]

    import jax
    import jax.numpy as jnp
    from concourse.bass2jax import (
        install_neuronx_cc_hook, partition_id_tensor, _bass_exec_p)
    from jax.sharding import Mesh, PartitionSpec, NamedSharding
    from jax.experimental.shard_map import shard_map

    nc = _get_program(nimg)
    install_neuronx_cc_hook()

    partition_name = (nc.partition_id_tensor.name
                      if nc.partition_id_tensor else None)
    in_names, out_names, out_avals, zero_shapes = [], [], [], []
    for alloc in nc.m.functions[0].allocations:
        if not isinstance(alloc, mybir.MemoryLocationSet):
            continue
        name = alloc.memorylocations[0].name
        if alloc.kind == "ExternalInput":
            if name != partition_name:
                in_names.append(name)
        elif alloc.kind == "ExternalOutput":
            shape = tuple(alloc.tensor_shape)
            dtype = mybir.dt.np(alloc.dtype)
            out_names.append(name)
            out_avals.append(jax.core.ShapedArray(shape, dtype))
            zero_shapes.append((shape, dtype))
    n_params = len(in_names)
    n_outs = len(out_names)
    all_in_names = list(in_names) + list(out_names)
    if partition_name is not None:
        all_in_names.append(partition_name)

    def _body(*args):
        operands = list(args)
        if partition_name is not None:
            operands.append(partition_id_tensor())
        outs = _bass_exec_p.bind(
            *operands,
            out_avals=tuple(out_avals),
            in_names=tuple(all_in_names),
            out_names=tuple(out_names),
            lowering_input_output_aliases=(),
            sim_require_finite=True,
            sim_require_nnan=True,
            nc=nc,
        )
        return tuple(outs)

    devices = jax.devices()[:N_CORES]
    mesh = Mesh(np.asarray(devices), ("core",))
    sh = NamedSharding(mesh, PartitionSpec("core"))
    donate = tuple(range(n_params, n_params + n_outs))
    sharded = jax.jit(
        shard_map(_body, mesh=mesh,
                  in_specs=(PartitionSpec("core"),) * (n_params + n_outs),
                  out_specs=(PartitionSpec("core"),) * n_outs,
                  check_rep=False),
        donate_argnums=donate, keep_unused=True)
    zeros_fn = jax.jit(
        lambda: tuple(
            jnp.zeros((N_CORES * s[0], *s[1:]), d) for s, d in zero_shapes),
        out_shardings=(sh,) * n_outs)

    def run(in_maps):
        concat_in = [
            np.concatenate([np.asarray(in_maps[c][nm])
                            for c in range(N_CORES)], axis=0)
            for nm in in_names
        ]
        dev_in = [jax.device_put(a, sh) for a in concat_in]
        outs = sharded(*dev_in, *zeros_fn())
        oi = out_names.index("out")
        return np.asarray(outs[oi])

    _runner_cache[nimg] = run
    return run


def _stage_x(x_core):
    """[n, CIN, H, W] f32 -> host im2col fold: [n, K1, NPIX] bf16 where
    row kw*15 + kh*5 + c at column (r, col) holds the zero-padded input
    value x[c, r + kh - 1, col + kw - 1]; kw=3 is the extra tap column
    the odd pixel of each packed pair needs."""
    n = x_core.shape[0]
    xp = np.zeros((n, CIN, H + 2, W + 3), dtype=np.float32)
    xp[:, :, 1:1 + H, 1:1 + W] = x_core
    x9 = np.empty((n, K1, NPIX), dtype=np.float32)
    for kw in range(4):
        for kh in range(3):
            base = kw * 15 + kh * CIN
            x9[:, base:base + CIN] = xp[:, :, kh:kh + H, kw:kw + W].reshape(
                n, CIN, NPIX)
    return _cast_bf16(x9)


def make_in_maps(x, w0, b0, w1, b1, w2, b2, w3, b3):
    """Shard the full inputs into the 8 per-core input maps."""
    consts = _prep_consts(w0, b0, w1, b1, w2, b2, w3, b3)
    bpc = B // N_CORES  # batches per core
    in_maps = []
    for c in range(N_CORES):
        xs = _stage_x(
            np.asarray(x)[c * bpc:(c + 1) * bpc].reshape(
                bpc * T, CIN, H, W))
        in_maps.append({"xin": xs, **consts})
    return in_maps


def kernel(x, w0, b0, w1, b1, w2, b2, w3, b3):
    nimg = (B // N_CORES) * T
    run = _get_runner(nimg)
    in_maps = make_in_maps(x, w0, b0, w1, b1, w2, b2, w3, b3)
    glob = run(in_maps)  # [8*nimg, 128, 4096] bf16
    bpc = B // N_CORES
    out = glob.reshape(N_CORES * bpc, T, CH[3], H, W).reshape(
        B, T, CH[3], H, W)
    return np.ascontiguousarray(out.astype(np.float32))
